# revision 29
# baseline (speedup 1.0000x reference)
"""Trainium2 Bass kernel for nn_Attn_Pred_Model (sparse_attention).

Math (per batch b, channel c):
    decay[t] = sum_{i=0}^{P-1} alpha * beta**i * x[t-i-1]        (P = past_steps)
    out[s,c] = (decay + pos_bias_fwd[c] + pos_bias_bwd[arange2[s,c]]) * mask[s,c]

Mapping:
  The causal exponential conv along S is a banded lower-triangular matmul.
  S goes on the contraction/partition axis, (channel, batch) on the moving
  free axis, processing S in 128-row chunks:
      out_chunk = Wprev.T @ x_prev_chunk + Wdiag.T @ x_chunk
  Both weight matrices are constant across chunks and batches (Wprev's rows
  0-63 are zero; keeping K=128 makes every matmul a dense full-array op,
  which measured faster than K=64/K=2 sub-array matmuls).

  With S = NB*NB and bucket stride NB, arange2 and mask are constant within
  64-row s-blocks: the gathered bias reduces to one row per (block, channel)
  added on the vector engine during the PSUM->SBUF copy, and the causal
  mask becomes a per-(chunk, channel) structure. Channels are placed OUTER
  in the free dim so masked-out channels form contiguous spans:
    - x channels c >= 2t+4 feed no live output -> not loaded, not computed;
    - output channels c > j of block j are never written; the PJRT-donated
      output buffer is zero-initialized, which supplies the masked zeros.

  x and the weights are DMA'd as fp16 (the PE multiplies fp32 at ~tf32
  precision anyway, measured no worse than float32r); accumulation is fp32
  in PSUM, the bias is added in fp32 on the vector engine during the
  PSUM->SBUF copy, and the output is stored fp16 (quantization ~5e-4 of the
  output scale) to halve write traffic. The kernel is HBM-bandwidth-bound
  with all 8 cores active.

Sharding: data-parallel over the batch dim across 8 cores (16 batches each).
Host side only reshuffles layout (B,S,C)->(S,C,B) and builds the tiny
(<=1MB) weight/bias tables; all O(B*S*C) compute runs on device.
"""

import numpy as np
from contextlib import ExitStack

import concourse.tile as tile
from concourse import bacc, mybir
from concourse.bass_utils import run_bass_kernel_spmd

N_CORES = 8
NB = 64            # channels / num buckets
CHUNK = 128        # s-rows per chunk (PE contraction tile)
CW = 32            # channels per free-group (CW * BL = 512 = fp32 PSUM bank)


# ---------------------------------------------------------------- device code

def _kernel_body(ctx, tc, aps, S, BL, repeats, act, out_swdge=False):
    """act[j] = number of active (mask=1) channels in 64-row block j."""
    nc = tc.nc
    nchunk = S // CHUNK
    nt = (NB + CW - 1) // CW   # free-groups per chunk (2)
    fw = CW * BL               # free width per group (512)

    consts = ctx.enter_context(tc.tile_pool(name="consts", bufs=1))
    xpool = ctx.enter_context(tc.tile_pool(name="xchunks", bufs=8))
    opool = ctx.enter_context(tc.tile_pool(name="outs", bufs=8))
    ppool = ctx.enter_context(tc.tile_pool(name="psum", bufs=8, space="PSUM"))

    f32 = mybir.dt.float32
    f16 = mybir.dt.float16

    wdiag_sb = consts.tile([128, 128], f16)
    nc.sync.dma_start(wdiag_sb[:], aps["wdiag"])
    wprev_sb = consts.tile([128, 128], f16)
    nc.sync.dma_start(wprev_sb[:], aps["wprev"])
    # biast[p, t*NB + c] = bias of (64-row block 2t + p//64, channel c)
    biast_sb = consts.tile([128, nchunk * NB], f32)
    nc.sync.dma_start(biast_sb[:], aps["biast"])

    x_ap = aps["x"]    # (S, NB, BL) fp16
    y_ap = aps["y"]    # (S, NB, BL) fp16

    out_dma_engine = nc.gpsimd if out_swdge else nc.scalar

    def one_pass():
        prev = None
        prev_ax = 0
        for t in range(nchunk):
            a0 = act[2 * t]           # active channels, rows [0,64)
            a1 = act[2 * t + 1]       # active channels, rows [64,128)
            # x channels needed by live outputs of chunks t and t+1
            a_next = act[2 * t + 3] if 2 * t + 3 < 2 * nchunk else 0
            ax = min(max(a1, a_next), NB)
            xt = xpool.tile([128, NB * BL], f16, tag="x")
            if ax > 0:
                nc.sync.dma_start(
                    xt[:, :ax * BL].rearrange("p (c b) -> p c b", b=BL),
                    x_ap[t * CHUNK:(t + 1) * CHUNK, :ax],
                )
            ot = opool.tile([128, NB * BL], f16, tag="o")
            groups = []
            for g in range(nt):
                c_lo = g * CW
                c_hi = min(a1, c_lo + CW)
                if c_hi <= c_lo:
                    continue
                ps = ppool.tile([128, fw], f32, name="ps", tag="ps")
                groups.append((c_lo, c_hi, ps))
            # same-weight matmuls back-to-back (only full-array K=128;
            # wprev's rows 0-63 are zero coefficients)
            for c_lo, c_hi, ps in groups:
                n = (c_hi - c_lo) * BL
                nc.tensor.matmul(
                    ps[:, :n],
                    wdiag_sb[:],
                    xt[:, c_lo * BL:c_hi * BL],
                    start=True, stop=(prev is None or prev_ax < c_hi),
                )
            for c_lo, c_hi, ps in groups:
                if prev is not None and prev_ax >= c_hi:
                    n = (c_hi - c_lo) * BL
                    nc.tensor.matmul(
                        ps[:, :n],
                        wprev_sb[:],
                        prev[:, c_lo * BL:c_hi * BL],
                        start=False, stop=True,
                    )
            for c_lo, c_hi, ps in groups:
                n = (c_hi - c_lo) * BL
                # bias added during the PSUM->SBUF copy (fp32, exact)
                b3 = (biast_sb[:, t * NB + c_lo:t * NB + c_hi]
                      .rearrange("p (c one) -> p c one", one=1)
                      .broadcast_to((128, c_hi - c_lo, BL)))
                nc.vector.tensor_add(
                    ot[:, c_lo * BL:c_hi * BL].rearrange(
                        "p (c b) -> p c b", b=BL),
                    ps[:, :n].rearrange("p (c b) -> p c b", b=BL),
                    b3,
                )
            # zero the strip dead in rows [0,64) but inside the written span
            if a0 < a1:
                nc.gpsimd.memset(ot[0:64, a0 * BL:a1 * BL], 0.0)
            # one output DMA per chunk covering all live channels [0, a1)
            if a1 > 0:
                out_dma_engine.dma_start(
                    y_ap[t * CHUNK:(t + 1) * CHUNK, 0:a1],
                    ot[:, :a1 * BL].rearrange("p (c b) -> p c b", b=BL),
                )
            prev = xt
            prev_ax = ax

    if repeats == 1:
        one_pass()
    else:
        from concourse.engine_type import EngineType
        with tc.For_i(0, repeats, 1,
                      hint_engines=(EngineType.PE, EngineType.DVE,
                                    EngineType.Activation, EngineType.SP)):
            one_pass()


_NC_CACHE = {}


def _build_nc(S, BL, repeats, act):
    key = (S, BL, repeats, tuple(act))
    if key in _NC_CACHE:
        return _NC_CACHE[key]
    f32 = mybir.dt.float32
    f16 = mybir.dt.float16
    nchunk = S // CHUNK
    nc = bacc.Bacc("TRN2", target_bir_lowering=False, debug=False)
    aps = {
        "x": nc.dram_tensor("x", (S, NB, BL), f16, kind="ExternalInput").ap(),
        "wdiag": nc.dram_tensor("wdiag", (128, 128), f16,
                                kind="ExternalInput").ap(),
        "wprev": nc.dram_tensor("wprev", (128, 128), f16,
                                kind="ExternalInput").ap(),
        "biast": nc.dram_tensor(
            "biast", (128, nchunk * NB), f32, kind="ExternalInput").ap(),
        "y": nc.dram_tensor("y", (S, NB, BL), f16, kind="ExternalOutput").ap(),
    }
    with tile.TileContext(nc) as tc:
        with ExitStack() as ctx:
            _kernel_body(ctx, tc, aps, S, BL, repeats, act)
    nc.compile()
    _NC_CACHE[key] = nc
    return nc


# ------------------------------------------------------------------ host prep

def _coeff(alpha, beta, past_steps):
    """coeff[d-1] = weight of x[t-d] in decay[t], d = 1..64."""
    d = np.arange(1, 65, dtype=np.float64)
    c = np.where(d <= past_steps, float(alpha) * float(beta) ** (d - 1), 0.0)
    return c.astype(np.float32)


def _weights(alpha, beta, past_steps):
    c = np.zeros(256, dtype=np.float32)
    c[1:65] = _coeff(alpha, beta, past_steps)

    k = np.arange(128)[:, None]
    m = np.arange(128)[None, :]
    d_diag = m - k          # s_out=(r0+m), s_in=(r0+k)
    d_prev = m + 128 - k    # s_in = r0-128+k
    wdiag = np.where((d_diag >= 1) & (d_diag <= 64), c[np.clip(d_diag, 0, 255)], 0.0)
    wprev = np.where((d_prev >= 1) & (d_prev <= 64), c[np.clip(d_prev, 0, 255)], 0.0)
    return wdiag.astype(np.float16), wprev.astype(np.float16)


def _tables(pos_bias_fwd, pos_bias_bwd, arange2, mask, S, BL):
    """biast (128, nchunk*NB) fp32 and act[nblk].

    Relies on arange2/mask being constant within each 64-row s-block
    (structural: arange2[s,c] = ((s - c*NB) % S)//NB, mask = tril blocks)
    and on mask being a prefix of ones along channels in each block."""
    nchunk = S // CHUNK
    nblk = S // 64
    a2 = np.asarray(arange2)
    blk = a2.reshape(nblk, 64, NB)
    assert (blk == blk[:, :1, :]).all(), "arange2 not block-constant"
    mk = np.asarray(mask, dtype=np.float32)
    mblk = mk.reshape(nblk, 64, NB)
    assert (mblk == mblk[:, :1, :]).all(), "mask not block-constant"
    act = mblk[:, 0, :].sum(axis=1).astype(np.int64)
    pref = np.arange(NB)[None, :] < act[:, None]
    assert (mblk[:, 0, :] == pref).all(), "mask not a channel-prefix"

    B = np.asarray(pos_bias_fwd)[0][None, :] + np.asarray(pos_bias_bwd)[0][blk[:, 0, :]]
    B = B.astype(np.float32).reshape(nchunk, 2, NB)
    # biast[p, t*NB + c] = B[t, p//64, c]
    biast = np.zeros((128, nchunk * NB), dtype=np.float32)
    biast[:64, :] = np.tile(B[:, 0, :].reshape(1, nchunk * NB), (64, 1))
    biast[64:, :] = np.tile(B[:, 1, :].reshape(1, nchunk * NB), (64, 1))
    return np.ascontiguousarray(biast), [int(v) for v in act]


def _make_in_maps(x, pos_bias_fwd, pos_bias_bwd, beta, alpha, arange2, mask,
                  past_steps, n_cores=N_CORES):
    B, S, C = x.shape
    assert C == NB and S % CHUNK == 0 and B % n_cores == 0
    BL = B // n_cores
    assert CW * BL <= 512
    P = int(np.asarray(past_steps))
    assert 1 <= P <= 64, f"past_steps={P} outside supported window"

    wdiag, wprev = _weights(np.asarray(alpha)[0], np.asarray(beta)[0], P)
    biast, act = _tables(pos_bias_fwd, pos_bias_bwd, arange2, mask, S, BL)

    common = {"wdiag": wdiag, "wprev": wprev, "biast": biast}
    x16 = x.astype(np.float16)
    in_maps = []
    for i in range(n_cores):
        xs = np.ascontiguousarray(
            x16[i * BL:(i + 1) * BL].transpose(1, 2, 0))   # (S, NB, BL)
        in_maps.append({"x": xs, **common})
    return in_maps, BL, act


def _run(x, pos_bias_fwd, pos_bias_bwd, beta, alpha, arange2, mask, past_steps,
         repeats=1):
    B, S, C = x.shape
    in_maps, BL, act = _make_in_maps(
        x, pos_bias_fwd, pos_bias_bwd, beta, alpha, arange2, mask, past_steps)
    nc = _build_nc(S, BL, repeats, act)
    res = run_bass_kernel_spmd(nc, in_maps, core_ids=list(range(N_CORES)))
    out = np.empty((B, S, C), dtype=np.float32)
    for i in range(N_CORES):
        out[i * BL:(i + 1) * BL] = res.results[i]["y"].transpose(
            2, 0, 1).astype(np.float32)
    return out


def kernel(x, pos_bias_fwd, pos_bias_bwd, beta, alpha, arange2, mask,
           past_steps, **_unused):
    x = np.asarray(x, dtype=np.float32)
    return _run(x, pos_bias_fwd, pos_bias_bwd, beta, alpha, arange2, mask,
                past_steps)
